# revision 1
# baseline (speedup 1.0000x reference)
"""Trainium2 Bass kernel for nn_CustomNodeGCN (GCN message passing).

Strategy (graph/data parallel across 8 NeuronCores):
  - Nodes are dealt to the 8 cores by degree rank (balanced); each core owns a
    contiguous range of "table rows" (core c -> rows [c*SL, (c+1)*SL)).
  - Per conv layer: each core computes u = dinv * (h @ W) for its node shard
    (feature-major matmul, PE transpose to node-major, ACT scale by dinv,
    cast to bf16), writes it to a DRAM bounce, AllGathers the full bf16 row
    table across cores, then dma_gathers the rows of its in-edges' sources
    into padded per-dst-node slots and segment-sums them with DVE reduces.
    The result is scaled by dinv[dst] (symmetric GCN norm folds into the two
    dinv scalings), transposed back to feature-major.
  - BatchNorm: per-feature stats via on-chip reductions + a tiny AllReduce;
    BN affine + ReLU fused into one ACT op (per-partition scale/bias in
    feature-major layout).  Conv bias before BN cancels exactly in BN.
  - dma_gather's int16 indices cap a gather window at 32768 table rows, so
    edges are bucketed by source core group (cores 0-4 -> window A at rows
    [0, 32767], cores 5-7 -> window B at rows [T-32768, T-1]); each dst tile
    has a padded slot block per window, pads point at known zero rows
    (dummy node slots whose table rows are zeroed via dinv=0).
"""

import math
import os

import numpy as np

# ---------------------------------------------------------------- config ----
N_NODES = 50000
E_EDGES = 800000
DIN = 128
H = 128
DOUT = 64
EPS = 1e-5

C = 8          # cores
P = 128        # partitions
A_CORES = 5    # cores 0..4 feed gather window A; 5..7 feed window B
GROUP_CAP = 128  # max sum of K rows (128-slot rows) per gather call group

_cache = {}


# ---------------------------------------------------------- preprocessing ---
def _preprocess(edge_index, n_nodes):
    """Host-side routing: node->core/slot assignment, padded gather slot
    lists (int16, two windows), dinv, group structure."""
    src = edge_index[0].astype(np.int64)
    dst = edge_index[1].astype(np.int64)
    N = n_nodes
    assert N % C == 0
    REAL = N // C
    TPC = REAL // P + 1          # tiles per core (>=1 dummy slot per core)
    SL = TPC * P                 # slots per core
    T_ROWS = C * SL
    WB_BASE = max(0, T_ROWS - 32768)
    assert (A_CORES + 3) * SL <= 32768 + 3 * SL or True
    assert A_CORES * SL <= 32768, "window A must cover cores 0..A_CORES-1"
    assert WB_BASE <= A_CORES * SL, "window B must cover cores A_CORES..7"

    deg = np.bincount(dst, minlength=N) + 1           # in-degree + self loop
    dinv = 1.0 / np.sqrt(deg.astype(np.float64))

    # core assignment: deal by total degree rank
    order0 = np.argsort(deg, kind="stable")
    rank0 = np.empty(N, np.int64)
    rank0[order0] = np.arange(N)
    core = rank0 % C

    # per-window in-degree (self loop included)
    srcA = core[src] < A_CORES
    selfA = core < A_CORES
    dA = np.bincount(dst[srcA], minlength=N) + selfA
    dB = np.bincount(dst[~srcA], minlength=N) + (~selfA)

    # within-core ordering: (dA asc, dB desc) packs tiles tightly
    local = np.empty(N, np.int64)
    for c in range(C):
        idx = np.where(core == c)[0]
        key = dA[idx] * 100000 - dB[idx]
        o = idx[np.argsort(key, kind="stable")]
        local[o] = np.arange(len(o))
    row = core * SL + local      # table row of each node

    # per-tile K (max over cores => same program on every core)
    KA = np.zeros((C, TPC), np.int64)
    KB = np.zeros((C, TPC), np.int64)
    for c in range(C):
        m = core == c
        t = local[m] // P
        np.maximum.at(KA[c], t, dA[m])
        np.maximum.at(KB[c], t, dB[m])
    KAg = KA.max(0)
    KBg = KB.max(0)

    # gather call groups: consecutive tiles, capped K sum
    groups = []
    g = []
    ksum = 0
    for t in range(TPC):
        kt = int(KAg[t] + KBg[t])
        if g and ksum + kt > GROUP_CAP:
            groups.append(g)
            g, ksum = [], 0
        g.append(t)
        ksum += kt
    if g:
        groups.append(g)

    # flat slot layout: per group: [A blocks of its tiles][B blocks]
    baseA = np.zeros(TPC, np.int64)   # k-row offset of tile t's A block
    baseB = np.zeros(TPC, np.int64)
    gmeta = []                        # (rowA0, nA_rows, rowB0, nB_rows)
    cur = 0
    for g in groups:
        a0 = cur
        for t in g:
            baseA[t] = cur
            cur += KAg[t]
        b0 = cur
        for t in g:
            baseB[t] = cur
            cur += KBg[t]
        gmeta.append((a0, b0 - a0, b0, cur - b0))
    TOTK = cur                        # total k-rows
    TOT_SLOTS = TOTK * P

    padA_row = 0 * SL + REAL          # core 0's first dummy slot (zero row)
    padB_row = A_CORES * SL + REAL    # core A_CORES's first dummy slot
    assert padA_row <= 32767
    assert WB_BASE <= padB_row < T_ROWS

    # per-core arrays
    xt_perm = np.zeros((C, SL), np.int64)     # slot -> original node (real)
    valid = np.zeros((C, SL), bool)
    for c in range(C):
        m = np.where(core == c)[0]
        xt_perm[c, local[m]] = m
        valid[c, local[m]] = True
    assert (valid[:, :REAL]).all() and not valid[:, REAL:].any()

    dinv_cols = np.zeros((C, P, TPC), np.float32)
    for c in range(C):
        loc = local[core == c]
        nodes = np.where(core == c)[0]
        dinv_cols[c, loc % P, loc // P] = dinv[nodes]

    # gather index arrays (int16), one per core
    # slot for (dst-core c, tile t, part p, window w, k) = (base_w[t]+k)*128+p
    idx16 = np.zeros((C, TOT_SLOTS), np.int16)
    # prefill pads
    padA_val = np.int16(padA_row)
    padB_val = np.int16(padB_row - WB_BASE)
    for (a0, na, b0, nb) in gmeta:
        idx16[:, a0 * P:(a0 + na) * P] = padA_val
        idx16[:, b0 * P:(b0 + nb) * P] = padB_val

    # edges incl self loops
    e_src = np.concatenate([src, np.arange(N)])
    e_dst = np.concatenate([dst, np.arange(N)])
    e_srow = row[e_src]
    e_A = core[e_src] < A_CORES
    e_c = core[e_dst]
    e_loc = local[e_dst]
    # k = rank of edge within its (dst, window) bucket
    okey = e_c * (SL * 2) + e_loc * 2 + (~e_A)
    eo = np.argsort(okey, kind="stable")
    sk = okey[eo]
    first = np.r_[True, sk[1:] != sk[:-1]]
    starts = np.where(first)[0]
    grp = np.cumsum(first) - 1
    k_in_grp = np.arange(len(eo)) - starts[grp]
    ks = np.empty(len(eo), np.int64)
    ks[eo] = k_in_grp

    t_of = e_loc // P
    p_of = e_loc % P
    base = np.where(e_A, baseA[t_of], baseB[t_of])
    slot = (base + ks) * P + p_of
    val = np.where(e_A, e_srow, e_srow - WB_BASE).astype(np.int16)
    idx16[e_c, slot] = val

    # SBUF idx layout: element j -> [j%16, j//16], 16-row block replicated
    # to all 128 partitions (one copy per Q7 core).
    idx_sb = idx16.reshape(C, TOT_SLOTS // 16, 16).transpose(0, 2, 1)
    idx_sb = np.tile(idx_sb, (1, 8, 1)).copy()

    return dict(
        REAL=REAL, TPC=TPC, SL=SL, T_ROWS=T_ROWS, WB_BASE=WB_BASE,
        KAg=KAg, KBg=KBg, groups=groups, gmeta=gmeta,
        baseA=baseA, baseB=baseB, TOTK=TOTK,
        xt_perm=xt_perm, dinv_cols=dinv_cols, idx_sb=idx_sb,
    )


# ------------------------------------------------------------- bass build ---
def _build(meta, n_real_total):
    import concourse.bacc as bacc
    import concourse.bass as bass
    import concourse.mybir as mybir
    import concourse.tile as tile
    from concourse.masks import make_identity

    f32 = mybir.dt.float32
    bf16 = mybir.dt.bfloat16
    i16 = mybir.dt.int16
    AF = mybir.ActivationFunctionType

    TPC, SL, T_ROWS = meta["TPC"], meta["SL"], meta["T_ROWS"]
    REAL = meta["REAL"]
    WB_BASE = meta["WB_BASE"]
    KAg, KBg = meta["KAg"], meta["KBg"]
    groups, gmeta = meta["groups"], meta["gmeta"]
    baseA, baseB = meta["baseA"], meta["baseB"]
    TOTK = meta["TOTK"]
    IDX_COLS = TOTK * P // 16
    WA_ROWS = min(T_ROWS, 32768)
    WB_ROWS = T_ROWS - WB_BASE

    nc = bacc.Bacc("TRN2", debug=False, num_devices=C, num_swdge_queues=4)

    # ---- I/O ----
    x_t = nc.dram_tensor("x_t", [P, SL], f32, kind="ExternalInput")
    idx_in = nc.dram_tensor("idx", [P, IDX_COLS], i16, kind="ExternalInput")
    dinv_in = nc.dram_tensor("dinv", [P, TPC], f32, kind="ExternalInput")
    w_names = ["pre_w1", "pre_w2", "cw0", "cw1", "cw2", "pw1"]
    w_in = {n: nc.dram_tensor(n, [H, H], f32, kind="ExternalInput") for n in w_names}
    w_in["pw2"] = nc.dram_tensor("pw2", [H, DOUT], f32, kind="ExternalInput")
    v_names = ["pre_b1", "pre_b2", "cb2", "bng0", "bnb0", "bng1", "bnb1", "pb1"]
    v_in = {n: nc.dram_tensor(n, [H, 1], f32, kind="ExternalInput") for n in v_names}
    v_in["pb2"] = nc.dram_tensor("pb2", [DOUT, 1], f32, kind="ExternalInput")
    out_t = nc.dram_tensor("out_t", [DOUT, SL], f32, kind="ExternalOutput")

    # matmul node chunks
    chunks = []
    o = 0
    while o < SL:
        w = min(512, SL - o)
        chunks.append((o, w))
        o += w

    with tile.TileContext(nc, num_cores=C) as tc:
        with (
            tc.tile_pool(name="persist", bufs=1) as pp,
            tc.tile_pool(name="gbuf", bufs=2) as gp,
            tc.tile_pool(name="work", bufs=3) as wp,
            tc.tile_pool(name="nodework", bufs=4) as nwp,
            tc.tile_pool(name="pmm", bufs=2, space="PSUM") as pmm,
            tc.tile_pool(name="ptp", bufs=4, space="PSUM") as ptp,
            tc.tile_pool(name="dram", bufs=1, space="DRAM") as dp,
        ):
            # ---- persistent tiles ----
            h_sb = pp.tile([P, SL], f32, tag="h")
            acc_sb = pp.tile([P, SL], f32, tag="acc")
            idx_sb = pp.tile([P, IDX_COLS], i16, tag="idx")
            dinv_sb = pp.tile([P, TPC], f32, tag="dinv")
            ident = pp.tile([P, P], f32, tag="ident")
            w_sb = {n: pp.tile(list(t.shape), f32, tag=f"w_{n}", name=f"w_{n}")
                    for n, t in w_in.items()}
            v_sb = {n: pp.tile(list(t.shape), f32, tag=f"v_{n}", name=f"v_{n}")
                    for n, t in v_in.items()}
            xt_sb = h_sb  # x is loaded into h and overwritten chunkwise

            shard_d = dp.tile([SL, H], bf16, tag="shard")
            table_ds = [dp.tile([T_ROWS, H], bf16, tag=f"table{i}",
                                name=f"table{i}")
                        for i in range(3)]
            tableS_ds = [dp.tile([T_ROWS, H], bf16, tag=f"tableS{i}",
                                 name=f"tableS{i}", addr_space="Shared")
                         for i in range(3)]
            st_in_d = dp.tile([P, 2], f32, tag="stin")
            st_out_ds = [dp.tile([P, 2], f32, tag=f"stout{i}",
                                 name=f"stout{i}")
                         for i in range(2)]

            # ---- loads ----
            nc.sync.dma_start(xt_sb[:], x_t[:, :])
            nc.sync.dma_start(idx_sb[:], idx_in[:, :])
            nc.sync.dma_start(dinv_sb[:], dinv_in[:, :])
            for n in w_sb:
                nc.sync.dma_start(w_sb[n][:], w_in[n][:, :])
            for n in v_sb:
                nc.sync.dma_start(v_sb[n][:], v_in[n][:, :])
            make_identity(nc, ident[:])

            # ---- pre-MLP (feature-major) ----
            for (o, w) in chunks:
                ps = pmm.tile([P, 512], f32, space="PSUM", tag="mm")
                nc.tensor.matmul(ps[:, :w], lhsT=w_sb["pre_w1"][:],
                                 rhs=xt_sb[:, o:o + w], start=True, stop=True)
                t0 = wp.tile([P, 512], f32, tag="u512")
                nc.scalar.activation(t0[:, :w], ps[:, :w], AF.Relu,
                                     bias=v_sb["pre_b1"][:, 0:1])
                ps2 = pmm.tile([P, 512], f32, space="PSUM", tag="mm")
                nc.tensor.matmul(ps2[:, :w], lhsT=w_sb["pre_w2"][:],
                                 rhs=t0[:, :w], start=True, stop=True)
                nc.scalar.activation(h_sb[:, o:o + w], ps2[:, :w], AF.Relu,
                                     bias=v_sb["pre_b2"][:, 0:1])
            nc.vector.memset(h_sb[:, REAL:SL], 0.0)

            # ---- conv layers ----
            n_layers = int(os.environ.get("GCN_LAYERS", "3"))
            skip_bn = bool(os.environ.get("GCN_SKIP_BN"))
            skip_gather = bool(os.environ.get("GCN_SKIP_GATHER"))
            layer_list = [("cw0", True), ("cw1", True), ("cw2", False)][:n_layers]
            for layer, (wn, has_bn) in enumerate(layer_list):
                has_bn = has_bn and not skip_bn
                # table shard build: u = dinv * (h @ W), node-major bf16
                for ci, (o, w) in enumerate(chunks):
                    ps = pmm.tile([P, 512], f32, space="PSUM", tag="mm")
                    nc.tensor.matmul(ps[:, :w], lhsT=w_sb[wn][:],
                                     rhs=h_sb[:, o:o + w], start=True, stop=True)
                    u0 = wp.tile([P, 512], f32, tag="u512")
                    nc.scalar.copy(u0[:, :w], ps[:, :w])
                    for b in range(w // P):
                        t = (o // P) + b
                        pt = ptp.tile([P, P], f32, space="PSUM", tag="tp")
                        nc.tensor.transpose(pt[:], u0[:, b * P:(b + 1) * P],
                                            ident[:])
                        tn = nwp.tile([P, P], bf16, tag="tnode")
                        nc.scalar.activation(tn[:], pt[:], AF.Copy,
                                             scale=dinv_sb[:, t:t + 1])
                        nc.sync.dma_start(shard_d[t * P:(t + 1) * P, :], tn[:])

                # replicate table across cores (AG into Shared, then a fast
                # local copy -- dma_gather cannot read Shared memory)
                table_d = table_ds[layer]
                table_s = tableS_ds[layer]
                nc.gpsimd.collective_compute(
                    "AllGather", mybir.AluOpType.bypass,
                    replica_groups=[list(range(C))],
                    ins=[shard_d[:, :].opt()],
                    outs=[table_s[:, :].opt()],
                )
                nc.sync.dma_start(table_d[:, :], table_s[:, :])

                # gather + segment-sum per tile
                tabA = table_d[0:WA_ROWS, :]
                tabB = table_d[WB_BASE:WB_BASE + WB_ROWS, :]
                for gi, g in enumerate(groups):
                    a0, na, b0, nb = gmeta[gi]
                    gb = gp.tile([P, GROUP_CAP, H], bf16, tag="gather")
                    if skip_gather:
                        nc.vector.memset(gb[:, 0:na + nb, :], 0.0)
                    else:
                        if na:
                            nc.gpsimd.dma_gather(
                                gb[:, 0:na, :], tabA, idx_sb[:, a0 * 8:(a0 + na) * 8],
                                na * P, na * P, H, single_packet=False,
                                queue_num=(2 * gi) % 4)
                        if nb:
                            nc.gpsimd.dma_gather(
                                gb[:, na:na + nb, :], tabB,
                                idx_sb[:, b0 * 8:(b0 + nb) * 8],
                                nb * P, nb * P, H, single_packet=False,
                                queue_num=(2 * gi + 1) % 4)
                    for t in g:
                        ka, kb = int(KAg[t]), int(KBg[t])
                        oa = int(baseA[t] - a0)
                        ob = int(baseB[t] - a0)
                        accn = nwp.tile([P, P], f32, tag="accn")
                        if ka and kb:
                            wa = nwp.tile([P, P], f32, tag="redA")
                            nc.vector.reduce_sum(
                                wa[:], gb[:, oa:oa + ka, :].rearrange(
                                    "p k f -> p f k"),
                                axis=mybir.AxisListType.X)
                            wb = nwp.tile([P, P], f32, tag="redB")
                            nc.vector.reduce_sum(
                                wb[:], gb[:, ob:ob + kb, :].rearrange(
                                    "p k f -> p f k"),
                                axis=mybir.AxisListType.X)
                            nc.vector.tensor_tensor(
                                out=accn[:], in0=wa[:], in1=wb[:],
                                op=mybir.AluOpType.add)
                        elif ka or kb:
                            sl = (gb[:, oa:oa + ka, :] if ka
                                  else gb[:, ob:ob + kb, :])
                            nc.vector.reduce_sum(
                                accn[:], sl.rearrange("p k f -> p f k"),
                                axis=mybir.AxisListType.X)
                        else:
                            nc.vector.memset(accn[:], 0.0)
                        # scale by dinv[dst], transpose to feature-major
                        acc2 = nwp.tile([P, P], f32, tag="accs")
                        nc.scalar.activation(acc2[:], accn[:], AF.Copy,
                                             scale=dinv_sb[:, t:t + 1])
                        pt = ptp.tile([P, P], f32, space="PSUM", tag="tp")
                        nc.tensor.transpose(pt[:], acc2[:], ident[:])
                        nc.scalar.copy(acc_sb[:, t * P:(t + 1) * P], pt[:])

                if has_bn:
                    gname = "bng0" if layer == 0 else "bng1"
                    bname = "bnb0" if layer == 0 else "bnb1"
                    # stats: sum over nodes (free dim) and sum of squares
                    ssum = pp.tile([P, 1], f32, tag="ssum")
                    nc.vector.reduce_sum(ssum[:], acc_sb[:, 0:SL],
                                         axis=mybir.AxisListType.X)
                    sq_parts = pp.tile([P, len(chunks)], f32, tag="sqp")
                    for ci, (o, w) in enumerate(chunks):
                        scr = wp.tile([P, 512], f32, tag="u512")
                        nc.scalar.activation(scr[:, :w], acc_sb[:, o:o + w],
                                             AF.Square,
                                             accum_out=sq_parts[:, ci:ci + 1])
                    ssq = pp.tile([P, 1], f32, tag="ssq")
                    nc.vector.reduce_sum(ssq[:], sq_parts[:],
                                         axis=mybir.AxisListType.X)
                    stat_sb = pp.tile([P, 2], f32, tag="stat")
                    nc.vector.tensor_copy(stat_sb[:, 0:1], ssum[:])
                    nc.vector.tensor_copy(stat_sb[:, 1:2], ssq[:])
                    st_out_d = st_out_ds[layer]
                    nc.sync.dma_start(st_in_d[:, :], stat_sb[:])
                    nc.gpsimd.collective_compute(
                        "AllReduce", mybir.AluOpType.add,
                        replica_groups=[list(range(C))],
                        ins=[st_in_d[:, :].opt()],
                        outs=[st_out_d[:, :].opt()],
                    )
                    stat_g = pp.tile([P, 2], f32, tag="statg")
                    nc.sync.dma_start(stat_g[:], st_out_d[:, :])
                    inv_n = 1.0 / float(n_real_total)
                    mean = pp.tile([P, 1], f32, tag="mean")
                    nc.scalar.mul(mean[:], stat_g[:, 0:1], inv_n)
                    ex2 = pp.tile([P, 1], f32, tag="ex2")
                    nc.scalar.mul(ex2[:], stat_g[:, 1:2], inv_n)
                    m2 = pp.tile([P, 1], f32, tag="m2")
                    nc.scalar.square(m2[:], mean[:])
                    var = pp.tile([P, 1], f32, tag="var")
                    nc.vector.tensor_tensor(out=var[:], in0=ex2[:], in1=m2[:],
                                            op=mybir.AluOpType.subtract)
                    vare = pp.tile([P, 1], f32, tag="vare")
                    nc.vector.tensor_scalar_add(vare[:], var[:], float(EPS))
                    sd = pp.tile([P, 1], f32, tag="sd")
                    nc.scalar.activation(sd[:], vare[:], AF.Sqrt)
                    rs = pp.tile([P, 1], f32, tag="rs")
                    nc.vector.reciprocal(rs[:], sd[:])
                    s_bn = pp.tile([P, 1], f32, tag="sbn")
                    nc.vector.tensor_tensor(out=s_bn[:], in0=rs[:],
                                            in1=v_sb[gname][:, 0:1],
                                            op=mybir.AluOpType.mult)
                    ms = pp.tile([P, 1], f32, tag="ms")
                    nc.vector.tensor_tensor(out=ms[:], in0=mean[:], in1=s_bn[:],
                                            op=mybir.AluOpType.mult)
                    t_bn = pp.tile([P, 1], f32, tag="tbn")
                    nc.vector.tensor_tensor(out=t_bn[:], in0=v_sb[bname][:, 0:1],
                                            in1=ms[:],
                                            op=mybir.AluOpType.subtract)
                    for (o, w) in chunks:
                        nc.scalar.activation(h_sb[:, o:o + w],
                                             acc_sb[:, o:o + w], AF.Relu,
                                             bias=t_bn[:, 0:1],
                                             scale=s_bn[:, 0:1])
                    nc.vector.memset(h_sb[:, REAL:SL], 0.0)
                else:
                    for (o, w) in chunks:
                        nc.scalar.activation(h_sb[:, o:o + w],
                                             acc_sb[:, o:o + w], AF.Identity,
                                             bias=v_sb["cb2"][:, 0:1])

            # ---- post-MLP ----
            for (o, w) in chunks:
                ps = pmm.tile([P, 512], f32, space="PSUM", tag="mm")
                nc.tensor.matmul(ps[:, :w], lhsT=w_sb["pw1"][:],
                                 rhs=h_sb[:, o:o + w], start=True, stop=True)
                t0 = wp.tile([P, 512], f32, tag="u512")
                nc.scalar.activation(t0[:, :w], ps[:, :w], AF.Relu,
                                     bias=v_sb["pb1"][:, 0:1])
                ps2 = pmm.tile([P, 512], f32, space="PSUM", tag="mm")
                nc.tensor.matmul(ps2[:DOUT, :w], lhsT=w_sb["pw2"][:],
                                 rhs=t0[:, :w], start=True, stop=True)
                ot = wp.tile([DOUT, 512], f32, tag="o512")
                nc.scalar.activation(ot[:, :w], ps2[:DOUT, :w], AF.Identity,
                                     bias=v_sb["pb2"][:, 0:1])
                nc.sync.dma_start(out_t[:, o:o + w], ot[:, :w])

    nc.compile()
    return nc


# ------------------------------------------------------------------ run -----
def _prepare_in_maps(inputs, meta):
    x = np.asarray(inputs["x"], np.float32)
    SL, REAL = meta["SL"], meta["REAL"]
    xt_perm, dinv_cols, idx_sb = meta["xt_perm"], meta["dinv_cols"], meta["idx_sb"]

    def rep(a):
        return np.ascontiguousarray(a.astype(np.float32))

    common = {
        "pre_w1": rep(inputs["pre_w1"]), "pre_w2": rep(inputs["pre_w2"]),
        "cw0": rep(inputs["conv_w0"]), "cw1": rep(inputs["conv_w1"]),
        "cw2": rep(inputs["conv_w2"]), "pw1": rep(inputs["post_w1"]),
        "pw2": rep(inputs["post_w2"]),
        "pre_b1": rep(inputs["pre_b1"]).reshape(H, 1),
        "pre_b2": rep(inputs["pre_b2"]).reshape(H, 1),
        "cb2": rep(inputs["conv_b2"]).reshape(H, 1),
        "bng0": rep(inputs["bn_g0"]).reshape(H, 1),
        "bnb0": rep(inputs["bn_b0"]).reshape(H, 1),
        "bng1": rep(inputs["bn_g1"]).reshape(H, 1),
        "bnb1": rep(inputs["bn_b1"]).reshape(H, 1),
        "pb1": rep(inputs["post_b1"]).reshape(H, 1),
        "pb2": rep(inputs["post_b2"]).reshape(DOUT, 1),
    }
    in_maps = []
    for c in range(C):
        xc = np.zeros((SL, x.shape[1]), np.float32)
        xc[:REAL] = x[xt_perm[c, :REAL]]
        m = dict(common)
        m["x_t"] = np.ascontiguousarray(xc.T)
        m["idx"] = np.ascontiguousarray(idx_sb[c])
        m["dinv"] = np.ascontiguousarray(dinv_cols[c])
        in_maps.append(m)
    return in_maps


def _assemble_output(results, meta, n_nodes):
    SL, REAL = meta["SL"], meta["REAL"]
    xt_perm = meta["xt_perm"]
    out = np.zeros((n_nodes, DOUT), np.float32)
    for c in range(C):
        oc = results[c]["out_t"]          # [DOUT, SL]
        out[xt_perm[c, :REAL]] = oc[:, :REAL].T
    return out


def _install_neff_disk_cache():
    """Cache walrus NEFF compiles on disk keyed by BIR hash (compiles take
    minutes; the BIR for a given graph/shape is deterministic)."""
    import hashlib
    import shutil

    import concourse.bass2jax as b2j
    import concourse.bass_utils as bu

    if getattr(b2j, "_gcn_neff_cache", False):
        return
    cache_dir = os.environ.get("GCN_NEFF_CACHE", "/tmp/gcn_neff_cache")
    os.makedirs(cache_dir, exist_ok=True)
    orig = bu.compile_bir_kernel

    def cached(bir_json, tmpdir, neff_name="file.neff"):
        h = hashlib.sha256(bir_json if isinstance(bir_json, bytes)
                           else bir_json.encode()).hexdigest()[:24]
        hit = os.path.join(cache_dir, f"{h}.neff")
        dst_dir = os.path.join(tmpdir, "sg00")
        if os.path.exists(hit):
            os.makedirs(dst_dir, exist_ok=True)
            dst = os.path.join(dst_dir, neff_name)
            shutil.copy(hit, dst)
            return dst
        neff = orig(bir_json, tmpdir, neff_name)
        try:
            shutil.copy(neff, hit)
        except OSError:
            pass
        return neff

    b2j.compile_bir_kernel = cached
    bu.compile_bir_kernel = cached
    b2j._gcn_neff_cache = True


def kernel(**inputs):
    from concourse.bass_utils import run_bass_kernel_spmd

    _install_neff_disk_cache()

    edge_index = np.asarray(inputs["edge_index"])
    n_nodes = int(np.asarray(inputs["x"]).shape[0])

    key = (n_nodes, edge_index.shape[1])
    if key not in _cache or os.environ.get("GCN_NO_CACHE"):
        meta = _preprocess(edge_index, n_nodes)
        nc = _build(meta, n_nodes)
        _cache[key] = (meta, nc, edge_index.tobytes())
    meta, nc, eb = _cache[key]
    if eb != edge_index.tobytes():
        meta = _preprocess(edge_index, n_nodes)
        nc = _build(meta, n_nodes)
        _cache[key] = (meta, nc, edge_index.tobytes())

    in_maps = _prepare_in_maps(inputs, meta)
    res = run_bass_kernel_spmd(
        nc, in_maps, core_ids=list(range(C)),
        trace=bool(os.environ.get("GCN_TRACE")),
    )
    out = _assemble_output(res.results, meta, n_nodes)
    if res.exec_time_ns is not None:
        kernel.last_exec_time_ns = res.exec_time_ns
    kernel.last_results = res
    return out


kernel.last_exec_time_ns = None
kernel.last_results = None

